# revision 33
# baseline (speedup 1.0000x reference)
"""Trainium2 Bass kernel for knn_interpolate(K=3) + ResMLP over B=8 point clouds.

Sharding: data-parallel, one cloud per NeuronCore (8 cores).

Per-core pipeline, software-pipelined over groups of 16 target tiles (128
targets each) so gathers and the MLP overlap the DVE-bound selection:
  A. scores[t,s] = pt.ps - |ps|^2/2 via bf16x2-split matmul (K=12; offline-
     verified exact-top-3 always inside approx top-8 with ample slack), then
     ACT shifts by -|pt|^2/2 (per-partition bias) so values become -d2/2 and
     fp16 keeps full relative resolution.
  B. DVE max/max_index -> top-8 candidate values+indices per target.
  C. dma_gather (<=1024 idxs/instr) of candidate coords; int16 wrapped-index
     tables built via a DRAM roundtrip + DVE interleave.
  D. exact fp32 d2 recompute in the reference op order ((dx^2+dy^2)+dz^2);
     exact top-3-of-8 + inverse-d2 weights.
  E. dma_gather of the 3 selected source feature rows; ACT pre-scale by
     normalized weights; PE transpose-accumulate -> interp^T (channel-major).
  F. channel-major ResMLP on tile pairs (N=256, float32r matmuls).
Host does layout-only prep (transposes / bf16 hi-lo splits / |ps|^2) and the
final unshard (channel-major -> row-major concat).
"""

import os
import sys

for _p in ("/opt/trn_rl_repo", "/root/.axon_site/_ro/trn_rl_repo"):
    if _p not in sys.path and os.path.isdir(_p):
        sys.path.insert(0, _p)

import numpy as np
import ml_dtypes

B = 8
NT = 8192
NS = 2048
C_TGT = 128
C_SRC = 256
C_HID = 256
C_OUT = 128
P = 128
NCAND = 8
K = 3

TT = NT // P          # 64 target tiles per core
GROUP = 16            # tiles per pipeline group
CH = 8                # tiles per coord-gather chunk  (8*128  = 1024 idxs)
FCH = 2               # tiles per feature-gather chunk (2*3*128 = 768 idxs)
FG = 8                # tiles per feature-gather buffer


def _bf16_split(x):
    hi = np.asarray(x, ml_dtypes.bfloat16)
    lo = np.asarray(x - hi.astype(np.float32), ml_dtypes.bfloat16)
    return hi, lo


def build_program(tt=TT):
    import concourse.bacc as bacc
    import concourse.mybir as mybir
    import concourse.tile as tile
    from concourse import bass

    f32 = mybir.dt.float32
    f32r = mybir.dt.float32r
    f16 = mybir.dt.float16
    bf16 = mybir.dt.bfloat16
    u16 = mybir.dt.uint16
    i16 = mybir.dt.int16
    i32 = mybir.dt.int32
    Alu = mybir.AluOpType
    Act = mybir.ActivationFunctionType

    nc = bacc.Bacc("TRN2", debug=False, num_devices=8)
    nt = tt * P
    G = min(GROUP, tt)
    n_grp = tt // G
    ch = min(CH, G)
    fg = min(FG, G)
    fch = min(FCH, fg)

    # ---- DRAM tensors ----
    d_lhsT = nc.dram_tensor("lhsT_pt", [12, nt], bf16, kind="ExternalInput").ap()
    d_rhs = nc.dram_tensor("rhs_ps", [12, NS], bf16, kind="ExternalInput").ap()
    d_ptT = nc.dram_tensor("ptT", [P, tt * 3], f32, kind="ExternalInput").ap()
    d_nptq = nc.dram_tensor("nptq", [P, tt], f32, kind="ExternalInput").ap()
    d_xtT = nc.dram_tensor("xtT", [C_TGT, nt], f32r, kind="ExternalInput").ap()
    d_pspad = nc.dram_tensor("ps_pad", [NS, 64], f32, kind="ExternalInput").ap()
    d_xs = nc.dram_tensor("xs", [NS, C_SRC], f32, kind="ExternalInput").ap()
    d_w1 = nc.dram_tensor("w1t", [P, 3 * 2 * P], f32r, kind="ExternalInput").ap()
    d_w2 = nc.dram_tensor("w2t", [P, 2 * P], f32r, kind="ExternalInput").ap()
    d_ws = nc.dram_tensor("wst", [P, 3 * P], f32r, kind="ExternalInput").ap()
    d_b1 = nc.dram_tensor("b1t", [P, 2], f32, kind="ExternalInput").ap()
    d_bo = nc.dram_tensor("bot", [P, 1], f32, kind="ExternalInput").ap()
    d_ident = nc.dram_tensor("ident", [P, P], f32, kind="ExternalInput").ap()
    d_out = nc.dram_tensor("outT", [C_OUT, nt], f32, kind="ExternalOutput").ap()
    d_scr_c = nc.dram_tensor("scr_c", [n_grp, P, NCAND * G], i16, kind="Internal").ap()
    d_scr_f = nc.dram_tensor("scr_f", [n_grp, P, G * K], i16, kind="Internal").ap()

    with tile.TileContext(nc) as tc:
        with (
            tc.tile_pool(name="const", bufs=1) as cpool,
            tc.tile_pool(name="sel", bufs=1) as selpool,
            tc.tile_pool(name="psum_s", bufs=1, space="PSUM") as pspool,
            tc.tile_pool(name="ssb", bufs=3) as spool,
            tc.tile_pool(name="gath", bufs=1) as gpool,
            tc.tile_pool(name="mlp", bufs=3) as mpool,
            tc.tile_pool(name="psum_m", bufs=1, space="PSUM") as psm,
        ):
            # ---- resident constants ----
            lhsT = cpool.tile([12, nt], bf16)
            nc.sync.dma_start(lhsT[:], d_lhsT)
            rhs = cpool.tile([12, NS], bf16)
            nc.sync.dma_start(rhs[:], d_rhs)
            ptT = cpool.tile([P, tt * 3], f32)
            nc.sync.dma_start(ptT[:], d_ptT)
            nptq = cpool.tile([P, tt], f32)
            nc.sync.dma_start(nptq[:], d_nptq)
            w1 = cpool.tile([P, 3 * 2 * P], f32r)
            nc.sync.dma_start(w1[:], d_w1)
            w2 = cpool.tile([P, 2 * P], f32r)
            nc.sync.dma_start(w2[:], d_w2)
            ws = cpool.tile([P, 3 * P], f32r)
            nc.sync.dma_start(ws[:], d_ws)
            b1 = cpool.tile([P, 2], f32)
            nc.sync.dma_start(b1[:], d_b1)
            bo = cpool.tile([P, 1], f32)
            nc.sync.dma_start(bo[:], d_bo)
            ident = cpool.tile([P, P], f32)
            nc.sync.dma_start(ident[:], d_ident)

            # ---- persistent per-core buffers ----
            m8 = selpool.tile([P, tt * NCAND], f16)
            idx8 = selpool.tile([P, tt, NCAND], u16)
            cpos = selpool.tile([P, tt, NCAND, 4], f32)
            snd2 = selpool.tile([P, tt * NCAND], f32)
            slots = selpool.tile([P, tt * NCAND], u16)
            wn = selpool.tile([P, tt, K], f32)
            sidx = selpool.tile([P, tt * K], i32)
            dx = selpool.tile([P, tt * NCAND], f32)
            dy = selpool.tile([P, tt * NCAND], f32)
            dz = selpool.tile([P, tt * NCAND], f32)
            t0b = selpool.tile([P, tt * NCAND], f32)
            t1b = selpool.tile([P, tt * NCAND], f32)
            w3 = selpool.tile([P, tt, K], f32)
            sumw = selpool.tile([P, tt], f32)
            rsum = selpool.tile([P, tt], f32)
            idx8f = selpool.tile([P, NCAND, tt], f32)
            accm = selpool.tile([P, tt * K], f32)
            maskt = selpool.tile([P, tt], u16)

            ptc3 = ptT.rearrange("p (t c) -> p t c", c=3)

            for g in range(n_grp):
                g0 = g * G
                # ============ Phase A: scores + top-8 (per tile) ============
                for i in range(g0, g0 + G):
                    s_sb = spool.tile([P, NS], f16, tag="s_sb")
                    for hh in range(2):
                        ps_s = pspool.tile([P, NS // 2], f32, tag="scores")
                        for n in range(NS // 2 // 512):
                            nc.tensor.matmul(
                                ps_s[:, n * 512:(n + 1) * 512],
                                lhsT=lhsT[:, i * P:(i + 1) * P],
                                rhs=rhs[:, hh * (NS // 2) + n * 512:hh * (NS // 2) + (n + 1) * 512],
                                start=True, stop=True,
                            )
                        # shift to -d2/2 so fp16 keeps relative resolution
                        nc.scalar.activation(
                            s_sb[:, hh * (NS // 2):(hh + 1) * (NS // 2)], ps_s[:],
                            Act.Identity, bias=nptq[:, i:i + 1],
                        )
                    nc.vector.max(out=m8[:, i * 8:(i + 1) * 8], in_=s_sb[:])
                    nc.vector.max_index(
                        out=idx8[:, i, :],
                        in_max=m8[:, i * 8:(i + 1) * 8],
                        in_values=s_sb[:],
                    )

                # ============ Phase C: candidate coord gather ============
                idx16 = spool.tile([P, NCAND * G], i16, tag="idx16")
                nc.vector.tensor_copy(
                    idx16.rearrange("p (j t) -> p j t", j=NCAND),
                    idx8.rearrange("p t j -> p j t")[:, :, g0:g0 + G],
                )
                nc.sync.dma_start(d_scr_c[g], idx16[:])
                xc = spool.tile([P, 8, NCAND * G], i16, tag="xc")
                scr_c_r = d_scr_c[g].rearrange("(r q) m -> q r m", q=16)
                for cc in range(8):
                    nc.sync.dma_start(xc[cc * 16:(cc + 1) * 16], scr_c_r)
                idx16c = spool.tile([P, NCAND, G, 8], i16, tag="idx16c")
                nc.vector.tensor_copy(
                    idx16c.rearrange("p j t r -> p (j t) r"),
                    xc.rearrange("p r m -> p m r"),
                )
                for j in range(NCAND):
                    for c8 in range(0, G, ch):
                        gath_c = gpool.tile([P, ch, 64], f32, tag="gc", bufs=3)
                        nc.gpsimd.dma_gather(
                            out_ap=gath_c[:],
                            in_ap=d_pspad,
                            idxs_ap=idx16c[:, j, c8:c8 + ch],
                            num_idxs=ch * P,
                            num_idxs_reg=ch * P,
                            elem_size=64,
                        )
                        nc.scalar.activation(
                            cpos[:, g0 + c8:g0 + c8 + ch, j, :],
                            gath_c[:, :, 0:4], Act.Copy,
                        )

                # ============ Phase D: exact refine ============
                gs8 = slice(g0 * NCAND, (g0 + G) * NCAND)
                cp = cpos[:, g0:g0 + G]                    # [P, G, 8, 4]
                for c, dst in ((0, dx), (1, dy), (2, dz)):
                    ptc = ptc3[:, g0:g0 + G, c:c + 1].to_broadcast([P, G, NCAND])
                    nc.vector.tensor_tensor(
                        out=dst.rearrange("p (t j) -> p t j", j=NCAND)[:, g0:g0 + G],
                        in0=cp[:, :, :, c], in1=ptc, op=Alu.subtract,
                    )
                nc.vector.tensor_tensor(t0b[:, gs8], dx[:, gs8], dx[:, gs8], op=Alu.mult)
                nc.vector.tensor_tensor(t1b[:, gs8], dy[:, gs8], dy[:, gs8], op=Alu.mult)
                nc.vector.tensor_tensor(t0b[:, gs8], t0b[:, gs8], t1b[:, gs8], op=Alu.add)
                nc.vector.tensor_tensor(t1b[:, gs8], dz[:, gs8], dz[:, gs8], op=Alu.mult)
                nc.vector.tensor_tensor(t0b[:, gs8], t0b[:, gs8], t1b[:, gs8], op=Alu.add)
                nd2 = dx  # reuse as -d2
                nc.vector.tensor_scalar(nd2[:, gs8], t0b[:, gs8], -1.0,
                                        scalar2=None, op0=Alu.mult)
                for i in range(g0, g0 + G):
                    nc.vector.max(out=snd2[:, i * 8:(i + 1) * 8],
                                  in_=nd2[:, i * 8:(i + 1) * 8])
                    nc.vector.max_index(
                        out=slots[:, i * 8:(i + 1) * 8],
                        in_max=snd2[:, i * 8:(i + 1) * 8],
                        in_values=nd2[:, i * 8:(i + 1) * 8],
                    )
                gsl = slice(g0, g0 + G)
                snd3 = snd2.rearrange("p (t c) -> p t c", c=NCAND)[:, gsl, 0:K]
                nc.vector.tensor_scalar(w3[:, gsl], snd3, -1.0, scalar2=None, op0=Alu.mult)
                nc.vector.reciprocal(w3[:, gsl], w3[:, gsl])
                nc.vector.tensor_tensor(sumw[:, gsl], w3[:, gsl, 0], w3[:, gsl, 1], op=Alu.add)
                nc.vector.tensor_tensor(sumw[:, gsl], sumw[:, gsl], w3[:, gsl, 2], op=Alu.add)
                nc.vector.reciprocal(rsum[:, gsl], sumw[:, gsl])
                for k in range(K):
                    nc.vector.tensor_tensor(wn[:, gsl, k], w3[:, gsl, k], rsum[:, gsl], op=Alu.mult)
                # slot -> source index
                nc.vector.tensor_copy(idx8f[:, :, gsl], idx8.rearrange("p t j -> p j t")[:, :, gsl])
                slotsf = t1b  # reuse as fp32 slots
                nc.vector.tensor_copy(slotsf[:, gs8], slots[:, gs8])
                gsK = slice(g0 * K, (g0 + G) * K)
                nc.vector.memset(accm[:, gsK], 0.0)
                for k in range(K):
                    for j in range(NCAND):
                        nc.vector.tensor_scalar(
                            maskt[:, gsl],
                            slotsf.rearrange("p (t c) -> p t c", c=NCAND)[:, gsl, k],
                            float(j), scalar2=None, op0=Alu.is_equal,
                        )
                        nc.vector.copy_predicated(
                            accm.rearrange("p (t c) -> p t c", c=K)[:, gsl, k],
                            maskt[:, gsl], idx8f[:, j, gsl],
                        )
                nc.vector.tensor_copy(sidx[:, gsK], accm[:, gsK])

                # ============ Phase E/F: feature gather + interp + MLP ============
                sidx16 = spool.tile([P, G * K], i16, tag="sidx16")
                nc.vector.tensor_copy(sidx16[:], sidx[:, gsK])
                nc.sync.dma_start(d_scr_f[g], sidx16[:])
                xf = spool.tile([P, 8, G * K], i16, tag="xf")
                scr_f_r = d_scr_f[g].rearrange("(r q) m -> q r m", q=16)
                for cc in range(8):
                    nc.sync.dma_start(xf[cc * 16:(cc + 1) * 16], scr_f_r)
                idx16f = spool.tile([P, G * K, 8], i16, tag="idx16f")
                nc.vector.tensor_copy(idx16f[:], xf.rearrange("p r m -> p m r"))
                for fg0 in range(0, G, fg):
                    gf = gpool.tile([P, fg * K, C_SRC], f32, tag="gf", bufs=2)
                    for c2 in range(0, fg, fch):
                        nc.gpsimd.dma_gather(
                            out_ap=gf[:, c2 * K:(c2 + fch) * K],
                            in_ap=d_xs,
                            idxs_ap=idx16f[:, (fg0 + c2) * K:(fg0 + c2 + fch) * K],
                            num_idxs=fch * K * P,
                            num_idxs_reg=fch * K * P,
                            elem_size=C_SRC,
                        )
                    for pp in range(0, fg, 2):      # tile pairs -> N=256 matmuls
                        i0 = g0 + fg0 + pp
                        it_lo = psm.tile([P, 2 * P], f32, tag="itlo", bufs=1)
                        it_hi = psm.tile([P, 2 * P], f32, tag="ithi", bufs=1)
                        gs_pair = []
                        for u in range(2):
                            i = i0 + u
                            ii = pp + u
                            gs = mpool.tile([P, K * C_SRC], f32, tag="gs")
                            for k in range(K):
                                # pre-scale by normalized weight (per-partition scale)
                                nc.scalar.activation(
                                    gs[:, k * C_SRC:(k + 1) * C_SRC],
                                    gf[:, ii * K + k, :],
                                    Act.Copy, scale=wn[:, i, k:k + 1],
                                )
                            gs_pair.append(gs)
                        for u, gs in enumerate(gs_pair):
                            for k in range(K):
                                nc.tensor.matmul(
                                    it_lo[:, u * P:(u + 1) * P],
                                    lhsT=gs[:, k * C_SRC:k * C_SRC + P],
                                    rhs=ident[:], is_transpose=True,
                                    start=(k == 0), stop=(k == K - 1),
                                )
                                nc.tensor.matmul(
                                    it_hi[:, u * P:(u + 1) * P],
                                    lhsT=gs[:, k * C_SRC + P:k * C_SRC + 2 * P],
                                    rhs=ident[:], is_transpose=True,
                                    start=(k == 0), stop=(k == K - 1),
                                )
                        ct0 = mpool.tile([P, 2 * P], f32r, tag="ct0")
                        nc.sync.dma_start(ct0[:], d_xtT[:, i0 * P:(i0 + 2) * P])
                        ct1 = mpool.tile([P, 2 * P], f32r, tag="ct1")
                        nc.scalar.activation(ct1[:], it_lo[:], Act.Copy)
                        ct2 = mpool.tile([P, 2 * P], f32r, tag="ct2")
                        nc.scalar.activation(ct2[:], it_hi[:], Act.Copy)
                        cts = (ct0, ct1, ct2)
                        ps_h = psm.tile([P, 2, 2 * P], f32, tag="ph", bufs=2)
                        for m in range(2):
                            for k in range(3):
                                nc.tensor.matmul(
                                    ps_h[:, m, :],
                                    lhsT=w1[:, (k * 2 + m) * P:(k * 2 + m + 1) * P],
                                    rhs=cts[k][:],
                                    start=(k == 0), stop=(k == 2),
                                )
                        hs = mpool.tile([P, 2, 2 * P], f32r, tag="hs")
                        for m in range(2):
                            nc.scalar.activation(
                                hs[:, m, :], ps_h[:, m, :],
                                Act.Relu, bias=b1[:, m:m + 1],
                            )
                        ps_o = psm.tile([P, 2 * P], f32, tag="po", bufs=1)
                        for k in range(2):
                            nc.tensor.matmul(
                                ps_o[:], lhsT=w2[:, k * P:(k + 1) * P],
                                rhs=hs[:, k, :], start=(k == 0), stop=False,
                            )
                        for k in range(3):
                            nc.tensor.matmul(
                                ps_o[:], lhsT=ws[:, k * P:(k + 1) * P],
                                rhs=cts[k][:], start=False, stop=(k == 2),
                            )
                        ot = mpool.tile([P, 2 * P], f32, tag="ot")
                        nc.scalar.activation(ot[:], ps_o[:], Act.Relu, bias=bo[:, 0:1])
                        nc.sync.dma_start(d_out[:, i0 * P:(i0 + 2) * P], ot[:])

    nc.compile()
    return nc


def host_prep(inputs, tt=TT):
    """Build the per-core input maps (layout-only host prep)."""
    nt = tt * P
    x_target = np.asarray(inputs["x_target"], np.float32)
    pos_target = np.asarray(inputs["pos_target"], np.float32)
    x_source = np.asarray(inputs["x_source"], np.float32)
    pos_source = np.asarray(inputs["pos_source"], np.float32)
    W1 = np.asarray(inputs["W1"], np.float32)
    b1 = np.asarray(inputs["b1"], np.float32)
    W2 = np.asarray(inputs["W2"], np.float32)
    b2 = np.asarray(inputs["b2"], np.float32)
    Ws = np.asarray(inputs["Ws"], np.float32)
    bs = np.asarray(inputs["bs"], np.float32)

    w1t = W1.reshape(3, P, 2, P).transpose(1, 0, 2, 3).reshape(P, 3 * 2 * P).copy()
    w2t = W2.reshape(2, P, P).transpose(1, 0, 2).reshape(P, 2 * P).copy()
    wst = Ws.reshape(3, P, P).transpose(1, 0, 2).reshape(P, 3 * P).copy()
    b1t = b1.reshape(2, P).T.copy()
    bot = (b2 + bs).reshape(P, 1).copy()
    ident = np.eye(P, dtype=np.float32)

    in_maps = []
    for c in range(B):
        pt = pos_target[c * NT:c * NT + nt]
        ps = pos_source[c * NS:(c + 1) * NS]
        a_hi, a_lo = _bf16_split(pt)
        b_hi, b_lo = _bf16_split(ps)
        q = -0.5 * (ps.astype(np.float64) ** 2).sum(-1)
        q = q.astype(np.float32)
        q_hi, q_lo = _bf16_split(q)
        one = np.ones(nt, ml_dtypes.bfloat16)
        zero = np.zeros(nt, ml_dtypes.bfloat16)
        lhsT = np.stack(
            [a_hi[:, 0], a_hi[:, 0], a_lo[:, 0],
             a_hi[:, 1], a_hi[:, 1], a_lo[:, 1],
             a_hi[:, 2], a_hi[:, 2], a_lo[:, 2],
             one, one, zero], axis=0)
        zs = np.zeros(NS, ml_dtypes.bfloat16)
        rhs = np.stack(
            [b_hi[:, 0], b_lo[:, 0], b_hi[:, 0],
             b_hi[:, 1], b_lo[:, 1], b_hi[:, 1],
             b_hi[:, 2], b_lo[:, 2], b_hi[:, 2],
             q_hi, q_lo, zs], axis=0)
        ptT = pt.reshape(tt, P, 3).transpose(1, 0, 2).reshape(P, tt * 3).copy()
        nptq = (-0.5 * (pt.astype(np.float32) ** 2).sum(-1, dtype=np.float32)).reshape(tt, P).T.copy()
        xtT = x_target[c * NT:c * NT + nt].T.copy()
        ps_pad = np.zeros((NS, 64), np.float32)
        ps_pad[:, :3] = ps
        xs = x_source[c * NS:(c + 1) * NS].copy()
        in_maps.append({
            "lhsT_pt": lhsT, "rhs_ps": rhs, "ptT": ptT, "nptq": nptq, "xtT": xtT,
            "ps_pad": ps_pad, "xs": xs,
            "w1t": w1t, "w2t": w2t, "wst": wst, "b1t": b1t, "bot": bot,
            "ident": ident,
        })
    return in_maps


_CACHED = {}
LAST_RESULT = None


def kernel(**inputs):
    global LAST_RESULT
    from concourse import bass_utils

    if "nc" not in _CACHED:
        _CACHED["nc"] = build_program(TT)
    nc = _CACHED["nc"]
    in_maps = host_prep(inputs, TT)
    res = bass_utils.run_bass_kernel_spmd(nc, in_maps, core_ids=list(range(B)))
    LAST_RESULT = res
    outs = []
    for c in range(B):
        outT = res.results[c]["outT"]
        outs.append(np.ascontiguousarray(outT.T))
    return np.concatenate(outs, axis=0)


# revision 35
# speedup vs baseline: 1.0163x; 1.0163x over previous
"""Trainium2 Bass kernel for knn_interpolate(K=3) + ResMLP over B=8 point clouds.

Sharding: data-parallel, one cloud per NeuronCore (8 cores).

Per-core pipeline, software-pipelined over groups of 16 target tiles (128
targets each) so gathers and the MLP overlap the DVE-bound selection:
  A. scores[t,s] = pt.ps - |ps|^2/2 via bf16x2-split matmul (K=12; offline-
     verified exact-top-3 always inside approx top-8 with ample slack), then
     ACT shifts by -|pt|^2/2 (per-partition bias) so values become -d2/2 and
     fp16 keeps full relative resolution.
  B. DVE max/max_index -> top-8 candidate values+indices per target.
  C. dma_gather (<=1024 idxs/instr) of candidate coords; int16 wrapped-index
     tables built via a DRAM roundtrip + DVE interleave.
  D. exact fp32 d2 recompute in the reference op order ((dx^2+dy^2)+dz^2);
     exact top-3-of-8 + inverse-d2 weights.
  E. dma_gather of the 3 selected source feature rows; ACT pre-scale by
     normalized weights; PE transpose-accumulate -> interp^T (channel-major).
  F. channel-major ResMLP on tile pairs (N=256, float32r matmuls).
Host does layout-only prep (transposes / bf16 hi-lo splits / |ps|^2) and the
final unshard (channel-major -> row-major concat).
"""

import os
import sys

for _p in ("/opt/trn_rl_repo", "/root/.axon_site/_ro/trn_rl_repo"):
    if _p not in sys.path and os.path.isdir(_p):
        sys.path.insert(0, _p)

import numpy as np
import ml_dtypes

B = 8
NT = 8192
NS = 2048
C_TGT = 128
C_SRC = 256
C_HID = 256
C_OUT = 128
P = 128
NCAND = 8
K = 3

TT = NT // P          # 64 target tiles per core
GROUP = 16            # tiles per pipeline group
CH = 8                # tiles per coord-gather chunk  (8*128  = 1024 idxs)
FCH = 2               # tiles per feature-gather chunk (2*3*128 = 768 idxs)
FG = 8                # tiles per feature-gather buffer


def _bf16_split(x):
    hi = np.asarray(x, ml_dtypes.bfloat16)
    lo = np.asarray(x - hi.astype(np.float32), ml_dtypes.bfloat16)
    return hi, lo


def build_program(tt=TT):
    import concourse.bacc as bacc
    import concourse.mybir as mybir
    import concourse.tile as tile
    from concourse import bass

    f32 = mybir.dt.float32
    f32r = mybir.dt.float32r
    f16 = mybir.dt.float16
    bf16 = mybir.dt.bfloat16
    u16 = mybir.dt.uint16
    i16 = mybir.dt.int16
    i32 = mybir.dt.int32
    Alu = mybir.AluOpType
    Act = mybir.ActivationFunctionType

    nc = bacc.Bacc("TRN2", debug=False, num_devices=8)
    nt = tt * P
    G = min(GROUP, tt)
    n_grp = tt // G
    ch = min(CH, G)
    fg = min(FG, G)
    fch = min(FCH, fg)

    # ---- DRAM tensors ----
    d_lhsT = nc.dram_tensor("lhsT_pt", [12, nt], bf16, kind="ExternalInput").ap()
    d_rhs = nc.dram_tensor("rhs_ps", [12, NS], bf16, kind="ExternalInput").ap()
    d_ptT = nc.dram_tensor("ptT", [P, tt * 3], f32, kind="ExternalInput").ap()
    d_nptq = nc.dram_tensor("nptq", [P, tt], f32, kind="ExternalInput").ap()
    d_xtT = nc.dram_tensor("xtT", [C_TGT, nt], f32r, kind="ExternalInput").ap()
    d_pspad = nc.dram_tensor("ps_pad", [NS, 64], f32, kind="ExternalInput").ap()
    d_xs = nc.dram_tensor("xs", [NS, C_SRC], f32, kind="ExternalInput").ap()
    d_w1 = nc.dram_tensor("w1t", [P, 3 * 2 * P], f32r, kind="ExternalInput").ap()
    d_w2 = nc.dram_tensor("w2t", [P, 2 * P], f32r, kind="ExternalInput").ap()
    d_ws = nc.dram_tensor("wst", [P, 3 * P], f32r, kind="ExternalInput").ap()
    d_b1 = nc.dram_tensor("b1t", [P, 2], f32, kind="ExternalInput").ap()
    d_bo = nc.dram_tensor("bot", [P, 1], f32, kind="ExternalInput").ap()
    d_ident = nc.dram_tensor("ident", [P, P], f32, kind="ExternalInput").ap()
    d_out = nc.dram_tensor("outT", [C_OUT, nt], f32, kind="ExternalOutput").ap()
    d_scr_c = nc.dram_tensor("scr_c", [n_grp, P, NCAND * G], i16, kind="Internal").ap()
    d_scr_f = nc.dram_tensor("scr_f", [n_grp, P, G * K], i16, kind="Internal").ap()

    with tile.TileContext(nc) as tc:
        with (
            tc.tile_pool(name="const", bufs=1) as cpool,
            tc.tile_pool(name="sel", bufs=1) as selpool,
            tc.tile_pool(name="psum_s", bufs=1, space="PSUM") as pspool,
            tc.tile_pool(name="ssb", bufs=3) as spool,
            tc.tile_pool(name="gath", bufs=1) as gpool,
            tc.tile_pool(name="mlp", bufs=3) as mpool,
            tc.tile_pool(name="psum_m", bufs=1, space="PSUM") as psm,
        ):
            # ---- resident constants ----
            lhsT = cpool.tile([12, nt], bf16)
            nc.sync.dma_start(lhsT[:], d_lhsT)
            rhs = cpool.tile([12, NS], bf16)
            nc.sync.dma_start(rhs[:], d_rhs)
            ptT = cpool.tile([P, tt * 3], f32)
            nc.sync.dma_start(ptT[:], d_ptT)
            nptq = cpool.tile([P, tt], f32)
            nc.sync.dma_start(nptq[:], d_nptq)
            w1 = cpool.tile([P, 3 * 2 * P], f32r)
            nc.sync.dma_start(w1[:], d_w1)
            w2 = cpool.tile([P, 2 * P], f32r)
            nc.sync.dma_start(w2[:], d_w2)
            ws = cpool.tile([P, 3 * P], f32r)
            nc.sync.dma_start(ws[:], d_ws)
            b1 = cpool.tile([P, 2], f32)
            nc.sync.dma_start(b1[:], d_b1)
            bo = cpool.tile([P, 1], f32)
            nc.sync.dma_start(bo[:], d_bo)
            ident = cpool.tile([P, P], f32)
            nc.sync.dma_start(ident[:], d_ident)

            # ---- persistent per-core buffers ----
            m8 = selpool.tile([P, tt * NCAND], f16)
            idx8 = selpool.tile([P, tt, NCAND], u16)
            cpos = selpool.tile([P, tt, NCAND, 4], f32)
            snd2 = selpool.tile([P, tt * NCAND], f32)
            slots = selpool.tile([P, tt * NCAND], u16)
            wn = selpool.tile([P, tt, K], f32)
            sidx = selpool.tile([P, tt * K], i32)
            dx = selpool.tile([P, tt * NCAND], f32)
            dy = selpool.tile([P, tt * NCAND], f32)
            dz = selpool.tile([P, tt * NCAND], f32)
            t0b = selpool.tile([P, tt * NCAND], f32)
            t1b = selpool.tile([P, tt * NCAND], f32)
            w3 = selpool.tile([P, tt, K], f32)
            sumw = selpool.tile([P, tt], f32)
            rsum = selpool.tile([P, tt], f32)
            idx8f = selpool.tile([P, NCAND, tt], f32)
            accm = selpool.tile([P, tt * K], f32)
            maskt = selpool.tile([P, tt], u16)
            mask3 = selpool.tile([P, tt, K], f32)

            ptc3 = ptT.rearrange("p (t c) -> p t c", c=3)

            for g in range(n_grp):
                g0 = g * G
                # ============ Phase A: scores + top-8 (per tile) ============
                for i in range(g0, g0 + G):
                    s_sb = spool.tile([P, NS], f16, tag="s_sb")
                    for hh in range(2):
                        ps_s = pspool.tile([P, NS // 2], f32, tag="scores")
                        for n in range(NS // 2 // 512):
                            nc.tensor.matmul(
                                ps_s[:, n * 512:(n + 1) * 512],
                                lhsT=lhsT[:, i * P:(i + 1) * P],
                                rhs=rhs[:, hh * (NS // 2) + n * 512:hh * (NS // 2) + (n + 1) * 512],
                                start=True, stop=True,
                            )
                        # shift to -d2/2 so fp16 keeps relative resolution
                        nc.scalar.activation(
                            s_sb[:, hh * (NS // 2):(hh + 1) * (NS // 2)], ps_s[:],
                            Act.Identity, bias=nptq[:, i:i + 1],
                        )
                    nc.vector.max(out=m8[:, i * 8:(i + 1) * 8], in_=s_sb[:])
                    nc.vector.max_index(
                        out=idx8[:, i, :],
                        in_max=m8[:, i * 8:(i + 1) * 8],
                        in_values=s_sb[:],
                    )

                # ============ Phase C: candidate coord gather ============
                idx16 = spool.tile([P, NCAND * G], i16, tag="idx16")
                nc.vector.tensor_copy(
                    idx16.rearrange("p (j t) -> p j t", j=NCAND),
                    idx8.rearrange("p t j -> p j t")[:, :, g0:g0 + G],
                )
                nc.sync.dma_start(d_scr_c[g], idx16[:])
                xc = spool.tile([P, 8, NCAND * G], i16, tag="xc")
                scr_c_r = d_scr_c[g].rearrange("(r q) m -> q r m", q=16)
                for cc in range(8):
                    nc.sync.dma_start(xc[cc * 16:(cc + 1) * 16], scr_c_r)
                idx16c = spool.tile([P, NCAND, G, 8], i16, tag="idx16c")
                nc.vector.tensor_copy(
                    idx16c.rearrange("p j t r -> p (j t) r"),
                    xc.rearrange("p r m -> p m r"),
                )
                for j in range(NCAND):
                    for c8 in range(0, G, ch):
                        gath_c = gpool.tile([P, ch, 64], f32, tag="gc", bufs=3)
                        nc.gpsimd.dma_gather(
                            out_ap=gath_c[:],
                            in_ap=d_pspad,
                            idxs_ap=idx16c[:, j, c8:c8 + ch],
                            num_idxs=ch * P,
                            num_idxs_reg=ch * P,
                            elem_size=64,
                        )
                        nc.scalar.activation(
                            cpos[:, g0 + c8:g0 + c8 + ch, j, :],
                            gath_c[:, :, 0:4], Act.Copy,
                        )

                # ============ Phase D: exact refine ============
                gs8 = slice(g0 * NCAND, (g0 + G) * NCAND)
                cp = cpos[:, g0:g0 + G]                    # [P, G, 8, 4]
                for c, dst in ((0, dx), (1, dy), (2, dz)):
                    ptc = ptc3[:, g0:g0 + G, c:c + 1].to_broadcast([P, G, NCAND])
                    nc.vector.tensor_tensor(
                        out=dst.rearrange("p (t j) -> p t j", j=NCAND)[:, g0:g0 + G],
                        in0=cp[:, :, :, c], in1=ptc, op=Alu.subtract,
                    )
                nc.vector.tensor_tensor(t0b[:, gs8], dx[:, gs8], dx[:, gs8], op=Alu.mult)
                nc.vector.tensor_tensor(t1b[:, gs8], dy[:, gs8], dy[:, gs8], op=Alu.mult)
                nc.vector.tensor_tensor(t0b[:, gs8], t0b[:, gs8], t1b[:, gs8], op=Alu.add)
                nc.vector.tensor_tensor(t1b[:, gs8], dz[:, gs8], dz[:, gs8], op=Alu.mult)
                nc.vector.tensor_tensor(t0b[:, gs8], t0b[:, gs8], t1b[:, gs8], op=Alu.add)
                nd2 = dx  # reuse as -d2
                nc.vector.tensor_scalar(nd2[:, gs8], t0b[:, gs8], -1.0,
                                        scalar2=None, op0=Alu.mult)
                for i in range(g0, g0 + G):
                    nc.vector.max(out=snd2[:, i * 8:(i + 1) * 8],
                                  in_=nd2[:, i * 8:(i + 1) * 8])
                    nc.vector.max_index(
                        out=slots[:, i * 8:(i + 1) * 8],
                        in_max=snd2[:, i * 8:(i + 1) * 8],
                        in_values=nd2[:, i * 8:(i + 1) * 8],
                    )
                gsl = slice(g0, g0 + G)
                snd3 = snd2.rearrange("p (t c) -> p t c", c=NCAND)[:, gsl, 0:K]
                nc.vector.tensor_scalar(w3[:, gsl], snd3, -1.0, scalar2=None, op0=Alu.mult)
                nc.vector.reciprocal(w3[:, gsl], w3[:, gsl])
                nc.vector.tensor_tensor(sumw[:, gsl], w3[:, gsl, 0], w3[:, gsl, 1], op=Alu.add)
                nc.vector.tensor_tensor(sumw[:, gsl], sumw[:, gsl], w3[:, gsl, 2], op=Alu.add)
                nc.vector.reciprocal(rsum[:, gsl], sumw[:, gsl])
                for k in range(K):
                    nc.vector.tensor_tensor(wn[:, gsl, k], w3[:, gsl, k], rsum[:, gsl], op=Alu.mult)
                # slot -> source index
                nc.vector.tensor_copy(idx8f[:, :, gsl], idx8.rearrange("p t j -> p j t")[:, :, gsl])
                slotsf = t1b  # reuse as fp32 slots
                nc.vector.tensor_copy(slotsf[:, gs8], slots[:, gs8])
                gsK = slice(g0 * K, (g0 + G) * K)
                nc.vector.memset(accm[:, gsK], 0.0)
                slots3 = slotsf.rearrange("p (t c) -> p t c", c=NCAND)[:, gsl, 0:K]
                accv = accm.rearrange("p (t c) -> p t c", c=K)[:, gsl]
                for j in range(NCAND):
                    nc.vector.tensor_scalar(
                        mask3[:, gsl], slots3, float(j),
                        scalar2=None, op0=Alu.is_equal,
                    )
                    srcb = idx8f[:, j, gsl].rearrange("p (t o) -> p t o", o=1)
                    nc.vector.tensor_tensor(
                        mask3[:, gsl], mask3[:, gsl],
                        srcb.to_broadcast([P, G, K]), op=Alu.mult,
                    )
                    nc.vector.tensor_tensor(
                        accv, accv, mask3[:, gsl], op=Alu.add,
                    )
                nc.vector.tensor_copy(sidx[:, gsK], accm[:, gsK])

                # ============ Phase E/F: feature gather + interp + MLP ============
                sidx16 = spool.tile([P, G * K], i16, tag="sidx16")
                nc.vector.tensor_copy(sidx16[:], sidx[:, gsK])
                nc.sync.dma_start(d_scr_f[g], sidx16[:])
                xf = spool.tile([P, 8, G * K], i16, tag="xf")
                scr_f_r = d_scr_f[g].rearrange("(r q) m -> q r m", q=16)
                for cc in range(8):
                    nc.sync.dma_start(xf[cc * 16:(cc + 1) * 16], scr_f_r)
                idx16f = spool.tile([P, G * K, 8], i16, tag="idx16f")
                nc.vector.tensor_copy(idx16f[:], xf.rearrange("p r m -> p m r"))
                for fg0 in range(0, G, fg):
                    gf = gpool.tile([P, fg * K, C_SRC], f32, tag="gf", bufs=2)
                    for c2 in range(0, fg, fch):
                        nc.gpsimd.dma_gather(
                            out_ap=gf[:, c2 * K:(c2 + fch) * K],
                            in_ap=d_xs,
                            idxs_ap=idx16f[:, (fg0 + c2) * K:(fg0 + c2 + fch) * K],
                            num_idxs=fch * K * P,
                            num_idxs_reg=fch * K * P,
                            elem_size=C_SRC,
                        )
                    for pp in range(0, fg, 2):      # tile pairs -> N=256 matmuls
                        i0 = g0 + fg0 + pp
                        it_lo = psm.tile([P, 2 * P], f32, tag="itlo", bufs=1)
                        it_hi = psm.tile([P, 2 * P], f32, tag="ithi", bufs=1)
                        gs_pair = []
                        for u in range(2):
                            i = i0 + u
                            ii = pp + u
                            gs = mpool.tile([P, K * C_SRC], f32, tag="gs")
                            for k in range(K):
                                # pre-scale by normalized weight (per-partition scale)
                                nc.scalar.activation(
                                    gs[:, k * C_SRC:(k + 1) * C_SRC],
                                    gf[:, ii * K + k, :],
                                    Act.Copy, scale=wn[:, i, k:k + 1],
                                )
                            gs_pair.append(gs)
                        for u, gs in enumerate(gs_pair):
                            for k in range(K):
                                nc.tensor.matmul(
                                    it_lo[:, u * P:(u + 1) * P],
                                    lhsT=gs[:, k * C_SRC:k * C_SRC + P],
                                    rhs=ident[:], is_transpose=True,
                                    start=(k == 0), stop=(k == K - 1),
                                )
                                nc.tensor.matmul(
                                    it_hi[:, u * P:(u + 1) * P],
                                    lhsT=gs[:, k * C_SRC + P:k * C_SRC + 2 * P],
                                    rhs=ident[:], is_transpose=True,
                                    start=(k == 0), stop=(k == K - 1),
                                )
                        ct0 = mpool.tile([P, 2 * P], f32r, tag="ct0")
                        nc.sync.dma_start(ct0[:], d_xtT[:, i0 * P:(i0 + 2) * P])
                        ct1 = mpool.tile([P, 2 * P], f32r, tag="ct1")
                        nc.scalar.activation(ct1[:], it_lo[:], Act.Copy)
                        ct2 = mpool.tile([P, 2 * P], f32r, tag="ct2")
                        nc.scalar.activation(ct2[:], it_hi[:], Act.Copy)
                        cts = (ct0, ct1, ct2)
                        ps_h = psm.tile([P, 2, 2 * P], f32, tag="ph", bufs=2)
                        for m in range(2):
                            for k in range(3):
                                nc.tensor.matmul(
                                    ps_h[:, m, :],
                                    lhsT=w1[:, (k * 2 + m) * P:(k * 2 + m + 1) * P],
                                    rhs=cts[k][:],
                                    start=(k == 0), stop=(k == 2),
                                )
                        hs = mpool.tile([P, 2, 2 * P], f32r, tag="hs")
                        for m in range(2):
                            nc.scalar.activation(
                                hs[:, m, :], ps_h[:, m, :],
                                Act.Relu, bias=b1[:, m:m + 1],
                            )
                        ps_o = psm.tile([P, 2 * P], f32, tag="po", bufs=1)
                        for k in range(2):
                            nc.tensor.matmul(
                                ps_o[:], lhsT=w2[:, k * P:(k + 1) * P],
                                rhs=hs[:, k, :], start=(k == 0), stop=False,
                            )
                        for k in range(3):
                            nc.tensor.matmul(
                                ps_o[:], lhsT=ws[:, k * P:(k + 1) * P],
                                rhs=cts[k][:], start=False, stop=(k == 2),
                            )
                        ot = mpool.tile([P, 2 * P], f32, tag="ot")
                        nc.scalar.activation(ot[:], ps_o[:], Act.Relu, bias=bo[:, 0:1])
                        nc.sync.dma_start(d_out[:, i0 * P:(i0 + 2) * P], ot[:])

    nc.compile()
    return nc


def host_prep(inputs, tt=TT):
    """Build the per-core input maps (layout-only host prep)."""
    nt = tt * P
    x_target = np.asarray(inputs["x_target"], np.float32)
    pos_target = np.asarray(inputs["pos_target"], np.float32)
    x_source = np.asarray(inputs["x_source"], np.float32)
    pos_source = np.asarray(inputs["pos_source"], np.float32)
    W1 = np.asarray(inputs["W1"], np.float32)
    b1 = np.asarray(inputs["b1"], np.float32)
    W2 = np.asarray(inputs["W2"], np.float32)
    b2 = np.asarray(inputs["b2"], np.float32)
    Ws = np.asarray(inputs["Ws"], np.float32)
    bs = np.asarray(inputs["bs"], np.float32)

    w1t = W1.reshape(3, P, 2, P).transpose(1, 0, 2, 3).reshape(P, 3 * 2 * P).copy()
    w2t = W2.reshape(2, P, P).transpose(1, 0, 2).reshape(P, 2 * P).copy()
    wst = Ws.reshape(3, P, P).transpose(1, 0, 2).reshape(P, 3 * P).copy()
    b1t = b1.reshape(2, P).T.copy()
    bot = (b2 + bs).reshape(P, 1).copy()
    ident = np.eye(P, dtype=np.float32)

    in_maps = []
    for c in range(B):
        pt = pos_target[c * NT:c * NT + nt]
        ps = pos_source[c * NS:(c + 1) * NS]
        a_hi, a_lo = _bf16_split(pt)
        b_hi, b_lo = _bf16_split(ps)
        q = -0.5 * (ps.astype(np.float64) ** 2).sum(-1)
        q = q.astype(np.float32)
        q_hi, q_lo = _bf16_split(q)
        one = np.ones(nt, ml_dtypes.bfloat16)
        zero = np.zeros(nt, ml_dtypes.bfloat16)
        lhsT = np.stack(
            [a_hi[:, 0], a_hi[:, 0], a_lo[:, 0],
             a_hi[:, 1], a_hi[:, 1], a_lo[:, 1],
             a_hi[:, 2], a_hi[:, 2], a_lo[:, 2],
             one, one, zero], axis=0)
        zs = np.zeros(NS, ml_dtypes.bfloat16)
        rhs = np.stack(
            [b_hi[:, 0], b_lo[:, 0], b_hi[:, 0],
             b_hi[:, 1], b_lo[:, 1], b_hi[:, 1],
             b_hi[:, 2], b_lo[:, 2], b_hi[:, 2],
             q_hi, q_lo, zs], axis=0)
        ptT = pt.reshape(tt, P, 3).transpose(1, 0, 2).reshape(P, tt * 3).copy()
        nptq = (-0.5 * (pt.astype(np.float32) ** 2).sum(-1, dtype=np.float32)).reshape(tt, P).T.copy()
        xtT = x_target[c * NT:c * NT + nt].T.copy()
        ps_pad = np.zeros((NS, 64), np.float32)
        ps_pad[:, :3] = ps
        xs = x_source[c * NS:(c + 1) * NS].copy()
        in_maps.append({
            "lhsT_pt": lhsT, "rhs_ps": rhs, "ptT": ptT, "nptq": nptq, "xtT": xtT,
            "ps_pad": ps_pad, "xs": xs,
            "w1t": w1t, "w2t": w2t, "wst": wst, "b1t": b1t, "bot": bot,
            "ident": ident,
        })
    return in_maps


_CACHED = {}
LAST_RESULT = None


def kernel(**inputs):
    global LAST_RESULT
    from concourse import bass_utils

    if "nc" not in _CACHED:
        _CACHED["nc"] = build_program(TT)
    nc = _CACHED["nc"]
    in_maps = host_prep(inputs, TT)
    res = bass_utils.run_bass_kernel_spmd(nc, in_maps, core_ids=list(range(B)))
    LAST_RESULT = res
    outs = []
    for c in range(B):
        outT = res.results[c]["outT"]
        outs.append(np.ascontiguousarray(outT.T))
    return np.concatenate(outs, axis=0)


# revision 36
# speedup vs baseline: 1.0587x; 1.0417x over previous
"""Trainium2 Bass kernel for knn_interpolate(K=3) + ResMLP over B=8 point clouds.

Sharding: data-parallel, one cloud per NeuronCore (8 cores).

Per-core pipeline, software-pipelined over groups of 16 target tiles (128
targets each) so gathers and the MLP overlap the DVE-bound selection:
  A. scores[t,s] = pt.ps - |ps|^2/2 via bf16x2-split matmul (K=12; offline-
     verified exact-top-3 always inside approx top-8 with ample slack), then
     ACT shifts by -|pt|^2/2 (per-partition bias) so values become -d2/2 and
     fp16 keeps full relative resolution.
  B. DVE max/max_index -> top-8 candidate values+indices per target.
  C. dma_gather (<=1024 idxs/instr) of candidate coords; int16 wrapped-index
     tables built via a DRAM roundtrip + DVE interleave.
  D. exact fp32 d2 recompute in the reference op order ((dx^2+dy^2)+dz^2);
     exact top-3-of-8 + inverse-d2 weights.
  E. dma_gather of the 3 selected source feature rows; ACT pre-scale by
     normalized weights; PE transpose-accumulate -> interp^T (channel-major).
  F. channel-major ResMLP on tile pairs (N=256, float32r matmuls).
Host does layout-only prep (transposes / bf16 hi-lo splits / |ps|^2) and the
final unshard (channel-major -> row-major concat).
"""

import os
import sys

for _p in ("/opt/trn_rl_repo", "/root/.axon_site/_ro/trn_rl_repo"):
    if _p not in sys.path and os.path.isdir(_p):
        sys.path.insert(0, _p)

import numpy as np
import ml_dtypes

B = 8
NT = 8192
NS = 2048
C_TGT = 128
C_SRC = 256
C_HID = 256
C_OUT = 128
P = 128
NCAND = 8
K = 3

TT = NT // P          # 64 target tiles per core
GROUP = 16            # tiles per pipeline group
CH = 8                # tiles per coord-gather chunk  (8*128  = 1024 idxs)
FCH = 2               # tiles per feature-gather chunk (2*3*128 = 768 idxs)
FG = 8                # tiles per feature-gather buffer


def _bf16_split(x):
    hi = np.asarray(x, ml_dtypes.bfloat16)
    lo = np.asarray(x - hi.astype(np.float32), ml_dtypes.bfloat16)
    return hi, lo


def build_program(tt=TT):
    import concourse.bacc as bacc
    import concourse.mybir as mybir
    import concourse.tile as tile
    from concourse import bass

    f32 = mybir.dt.float32
    f32r = mybir.dt.float32r
    f16 = mybir.dt.float16
    bf16 = mybir.dt.bfloat16
    u16 = mybir.dt.uint16
    i16 = mybir.dt.int16
    i32 = mybir.dt.int32
    Alu = mybir.AluOpType
    Act = mybir.ActivationFunctionType

    nc = bacc.Bacc("TRN2", debug=False, num_devices=8)
    nt = tt * P
    G = min(GROUP, tt)
    n_grp = tt // G
    ch = min(CH, G)
    fg = min(FG, G)
    fch = min(FCH, fg)

    # ---- DRAM tensors ----
    d_lhsT = nc.dram_tensor("lhsT_pt", [12, nt], bf16, kind="ExternalInput").ap()
    d_rhs = nc.dram_tensor("rhs_ps", [12, NS], bf16, kind="ExternalInput").ap()
    d_ptT = nc.dram_tensor("ptT", [P, tt * 3], f32, kind="ExternalInput").ap()
    d_nptq = nc.dram_tensor("nptq", [P, tt], f32, kind="ExternalInput").ap()
    d_xtT = nc.dram_tensor("xtT", [C_TGT, nt], f32r, kind="ExternalInput").ap()
    d_pspad = nc.dram_tensor("ps_pad", [NS, 64], f32, kind="ExternalInput").ap()
    d_xs = nc.dram_tensor("xs", [NS, C_SRC], f32, kind="ExternalInput").ap()
    d_w1 = nc.dram_tensor("w1t", [P, 3 * 2 * P], f32r, kind="ExternalInput").ap()
    d_w2 = nc.dram_tensor("w2t", [P, 2 * P], f32r, kind="ExternalInput").ap()
    d_ws = nc.dram_tensor("wst", [P, 3 * P], f32r, kind="ExternalInput").ap()
    d_b1 = nc.dram_tensor("b1t", [P, 2], f32, kind="ExternalInput").ap()
    d_bo = nc.dram_tensor("bot", [P, 1], f32, kind="ExternalInput").ap()
    d_ident = nc.dram_tensor("ident", [P, P], f32, kind="ExternalInput").ap()
    d_out = nc.dram_tensor("outT", [C_OUT, nt], f32, kind="ExternalOutput").ap()
    d_scr_c = nc.dram_tensor("scr_c", [n_grp, P, NCAND * G], i16, kind="Internal").ap()
    d_scr_f = nc.dram_tensor("scr_f", [n_grp, P, G * K], i16, kind="Internal").ap()

    with tile.TileContext(nc) as tc:
        with (
            tc.tile_pool(name="const", bufs=1) as cpool,
            tc.tile_pool(name="sel", bufs=1) as selpool,
            tc.tile_pool(name="psum_s", bufs=1, space="PSUM") as pspool,
            tc.tile_pool(name="ssb", bufs=3) as spool,
            tc.tile_pool(name="gath", bufs=1) as gpool,
            tc.tile_pool(name="mlp", bufs=3) as mpool,
            tc.tile_pool(name="psum_m", bufs=1, space="PSUM") as psm,
        ):
            # ---- resident constants ----
            lhsT = cpool.tile([12, nt], bf16)
            nc.sync.dma_start(lhsT[:], d_lhsT)
            rhs = cpool.tile([12, NS], bf16)
            nc.sync.dma_start(rhs[:], d_rhs)
            ptT = cpool.tile([P, tt * 3], f32)
            nc.sync.dma_start(ptT[:], d_ptT)
            nptq = cpool.tile([P, tt], f32)
            nc.sync.dma_start(nptq[:], d_nptq)
            w1 = cpool.tile([P, 3 * 2 * P], f32r)
            nc.sync.dma_start(w1[:], d_w1)
            w2 = cpool.tile([P, 2 * P], f32r)
            nc.sync.dma_start(w2[:], d_w2)
            ws = cpool.tile([P, 3 * P], f32r)
            nc.sync.dma_start(ws[:], d_ws)
            b1 = cpool.tile([P, 2], f32)
            nc.sync.dma_start(b1[:], d_b1)
            bo = cpool.tile([P, 1], f32)
            nc.sync.dma_start(bo[:], d_bo)
            ident = cpool.tile([P, P], f32)
            nc.sync.dma_start(ident[:], d_ident)

            # ---- persistent per-core buffers ----
            m8 = selpool.tile([P, tt * NCAND], f16)
            idx8 = selpool.tile([P, tt, NCAND], u16)
            cpos = selpool.tile([P, tt, NCAND, 4], f32)
            snd2 = selpool.tile([P, tt * NCAND], f32)
            slots = selpool.tile([P, tt * NCAND], u16)
            wn = selpool.tile([P, tt, K], f32)
            sidx = selpool.tile([P, tt * K], i32)
            dx = selpool.tile([P, tt * NCAND], f32)
            dy = selpool.tile([P, tt * NCAND], f32)
            dz = selpool.tile([P, tt * NCAND], f32)
            t0b = selpool.tile([P, tt * NCAND], f32)
            t1b = selpool.tile([P, tt * NCAND], f32)
            w3 = selpool.tile([P, tt, K], f32)
            sumw = selpool.tile([P, tt], f32)
            rsum = selpool.tile([P, tt], f32)
            idx8f = selpool.tile([P, NCAND, tt], f32)
            accm = selpool.tile([P, tt * K], f32)
            maskt = selpool.tile([P, tt], u16)
            mask3 = selpool.tile([P, tt, K], f32)

            ptc3 = ptT.rearrange("p (t c) -> p t c", c=3)

            for g in range(n_grp):
                g0 = g * G
                # ============ Phase A: scores + top-8 (per tile) ============
                for i in range(g0, g0 + G):
                    s_sb = spool.tile([P, NS], f16, tag="s_sb")
                    for hh in range(2):
                        ps_s = pspool.tile([P, NS // 2], f32, tag="scores")
                        for n in range(NS // 2 // 512):
                            nc.tensor.matmul(
                                ps_s[:, n * 512:(n + 1) * 512],
                                lhsT=lhsT[:, i * P:(i + 1) * P],
                                rhs=rhs[:, hh * (NS // 2) + n * 512:hh * (NS // 2) + (n + 1) * 512],
                                start=True, stop=True,
                            )
                        # shift to -d2/2 so fp16 keeps relative resolution
                        nc.scalar.activation(
                            s_sb[:, hh * (NS // 2):(hh + 1) * (NS // 2)], ps_s[:],
                            Act.Identity, bias=nptq[:, i:i + 1],
                        )
                    nc.vector.max(out=m8[:, i * 8:(i + 1) * 8], in_=s_sb[:])
                    nc.vector.max_index(
                        out=idx8[:, i, :],
                        in_max=m8[:, i * 8:(i + 1) * 8],
                        in_values=s_sb[:],
                    )

                # ============ Phase C: candidate coord gather ============
                idx16 = spool.tile([P, NCAND * G], i16, tag="idx16")
                nc.gpsimd.tensor_copy(
                    idx16.rearrange("p (j t) -> p j t", j=NCAND),
                    idx8.rearrange("p t j -> p j t")[:, :, g0:g0 + G],
                )
                nc.sync.dma_start(d_scr_c[g], idx16[:])
                xc = spool.tile([P, 8, NCAND * G], i16, tag="xc")
                scr_c_r = d_scr_c[g].rearrange("(r q) m -> q r m", q=16)
                for cc in range(8):
                    nc.sync.dma_start(xc[cc * 16:(cc + 1) * 16], scr_c_r)
                idx16c = spool.tile([P, NCAND, G, 8], i16, tag="idx16c")
                nc.gpsimd.tensor_copy(
                    idx16c.rearrange("p j t r -> p (j t) r"),
                    xc.rearrange("p r m -> p m r"),
                )
                for j in range(NCAND):
                    for c8 in range(0, G, ch):
                        gath_c = gpool.tile([P, ch, 64], f32, tag="gc", bufs=3)
                        nc.gpsimd.dma_gather(
                            out_ap=gath_c[:],
                            in_ap=d_pspad,
                            idxs_ap=idx16c[:, j, c8:c8 + ch],
                            num_idxs=ch * P,
                            num_idxs_reg=ch * P,
                            elem_size=64,
                        )
                        nc.scalar.activation(
                            cpos[:, g0 + c8:g0 + c8 + ch, j, :],
                            gath_c[:, :, 0:4], Act.Copy,
                        )

                # ============ Phase D: exact refine ============
                gs8 = slice(g0 * NCAND, (g0 + G) * NCAND)
                cp = cpos[:, g0:g0 + G]                    # [P, G, 8, 4]
                for c, dst in ((0, dx), (1, dy), (2, dz)):
                    ptc = ptc3[:, g0:g0 + G, c:c + 1].to_broadcast([P, G, NCAND])
                    nc.vector.tensor_tensor(
                        out=dst.rearrange("p (t j) -> p t j", j=NCAND)[:, g0:g0 + G],
                        in0=cp[:, :, :, c], in1=ptc, op=Alu.subtract,
                    )
                nc.vector.tensor_tensor(t0b[:, gs8], dx[:, gs8], dx[:, gs8], op=Alu.mult)
                nc.vector.tensor_tensor(t1b[:, gs8], dy[:, gs8], dy[:, gs8], op=Alu.mult)
                nc.vector.tensor_tensor(t0b[:, gs8], t0b[:, gs8], t1b[:, gs8], op=Alu.add)
                nc.vector.tensor_tensor(t1b[:, gs8], dz[:, gs8], dz[:, gs8], op=Alu.mult)
                nc.vector.tensor_tensor(t0b[:, gs8], t0b[:, gs8], t1b[:, gs8], op=Alu.add)
                nd2 = dx  # reuse as -d2
                nc.vector.tensor_scalar(nd2[:, gs8], t0b[:, gs8], -1.0,
                                        scalar2=None, op0=Alu.mult)
                for i in range(g0, g0 + G):
                    nc.vector.max(out=snd2[:, i * 8:(i + 1) * 8],
                                  in_=nd2[:, i * 8:(i + 1) * 8])
                    nc.vector.max_index(
                        out=slots[:, i * 8:(i + 1) * 8],
                        in_max=snd2[:, i * 8:(i + 1) * 8],
                        in_values=nd2[:, i * 8:(i + 1) * 8],
                    )
                gsl = slice(g0, g0 + G)
                snd3 = snd2.rearrange("p (t c) -> p t c", c=NCAND)[:, gsl, 0:K]
                nc.vector.tensor_scalar(w3[:, gsl], snd3, -1.0, scalar2=None, op0=Alu.mult)
                nc.vector.reciprocal(w3[:, gsl], w3[:, gsl])
                nc.vector.tensor_tensor(sumw[:, gsl], w3[:, gsl, 0], w3[:, gsl, 1], op=Alu.add)
                nc.vector.tensor_tensor(sumw[:, gsl], sumw[:, gsl], w3[:, gsl, 2], op=Alu.add)
                nc.vector.reciprocal(rsum[:, gsl], sumw[:, gsl])
                for k in range(K):
                    nc.vector.tensor_tensor(wn[:, gsl, k], w3[:, gsl, k], rsum[:, gsl], op=Alu.mult)
                # slot -> source index
                nc.gpsimd.tensor_copy(idx8f[:, :, gsl], idx8.rearrange("p t j -> p j t")[:, :, gsl])
                slotsf = t1b  # reuse as fp32 slots
                nc.gpsimd.tensor_copy(slotsf[:, gs8], slots[:, gs8])
                gsK = slice(g0 * K, (g0 + G) * K)
                nc.vector.memset(accm[:, gsK], 0.0)
                slots3 = slotsf.rearrange("p (t c) -> p t c", c=NCAND)[:, gsl, 0:K]
                accv = accm.rearrange("p (t c) -> p t c", c=K)[:, gsl]
                for j in range(NCAND):
                    nc.vector.tensor_scalar(
                        mask3[:, gsl], slots3, float(j),
                        scalar2=None, op0=Alu.is_equal,
                    )
                    srcb = idx8f[:, j, gsl].rearrange("p (t o) -> p t o", o=1)
                    nc.vector.tensor_tensor(
                        mask3[:, gsl], mask3[:, gsl],
                        srcb.to_broadcast([P, G, K]), op=Alu.mult,
                    )
                    nc.vector.tensor_tensor(
                        accv, accv, mask3[:, gsl], op=Alu.add,
                    )
                nc.gpsimd.tensor_copy(sidx[:, gsK], accm[:, gsK])

                # ============ Phase E/F: feature gather + interp + MLP ============
                sidx16 = spool.tile([P, G * K], i16, tag="sidx16")
                nc.gpsimd.tensor_copy(sidx16[:], sidx[:, gsK])
                nc.sync.dma_start(d_scr_f[g], sidx16[:])
                xf = spool.tile([P, 8, G * K], i16, tag="xf")
                scr_f_r = d_scr_f[g].rearrange("(r q) m -> q r m", q=16)
                for cc in range(8):
                    nc.sync.dma_start(xf[cc * 16:(cc + 1) * 16], scr_f_r)
                idx16f = spool.tile([P, G * K, 8], i16, tag="idx16f")
                nc.gpsimd.tensor_copy(idx16f[:], xf.rearrange("p r m -> p m r"))
                for fg0 in range(0, G, fg):
                    gf = gpool.tile([P, fg * K, C_SRC], f32, tag="gf", bufs=2)
                    for c2 in range(0, fg, fch):
                        nc.gpsimd.dma_gather(
                            out_ap=gf[:, c2 * K:(c2 + fch) * K],
                            in_ap=d_xs,
                            idxs_ap=idx16f[:, (fg0 + c2) * K:(fg0 + c2 + fch) * K],
                            num_idxs=fch * K * P,
                            num_idxs_reg=fch * K * P,
                            elem_size=C_SRC,
                        )
                    for pp in range(0, fg, 2):      # tile pairs -> N=256 matmuls
                        i0 = g0 + fg0 + pp
                        it_lo = psm.tile([P, 2 * P], f32, tag="itlo", bufs=1)
                        it_hi = psm.tile([P, 2 * P], f32, tag="ithi", bufs=1)
                        gs_pair = []
                        for u in range(2):
                            i = i0 + u
                            ii = pp + u
                            gs = mpool.tile([P, K * C_SRC], f32, tag="gs")
                            for k in range(K):
                                # pre-scale by normalized weight (per-partition scale)
                                nc.scalar.activation(
                                    gs[:, k * C_SRC:(k + 1) * C_SRC],
                                    gf[:, ii * K + k, :],
                                    Act.Copy, scale=wn[:, i, k:k + 1],
                                )
                            gs_pair.append(gs)
                        for u, gs in enumerate(gs_pair):
                            for k in range(K):
                                nc.tensor.matmul(
                                    it_lo[:, u * P:(u + 1) * P],
                                    lhsT=gs[:, k * C_SRC:k * C_SRC + P],
                                    rhs=ident[:], is_transpose=True,
                                    start=(k == 0), stop=(k == K - 1),
                                )
                                nc.tensor.matmul(
                                    it_hi[:, u * P:(u + 1) * P],
                                    lhsT=gs[:, k * C_SRC + P:k * C_SRC + 2 * P],
                                    rhs=ident[:], is_transpose=True,
                                    start=(k == 0), stop=(k == K - 1),
                                )
                        ct0 = mpool.tile([P, 2 * P], f32r, tag="ct0")
                        nc.sync.dma_start(ct0[:], d_xtT[:, i0 * P:(i0 + 2) * P])
                        ct1 = mpool.tile([P, 2 * P], f32r, tag="ct1")
                        nc.scalar.activation(ct1[:], it_lo[:], Act.Copy)
                        ct2 = mpool.tile([P, 2 * P], f32r, tag="ct2")
                        nc.scalar.activation(ct2[:], it_hi[:], Act.Copy)
                        cts = (ct0, ct1, ct2)
                        ps_h = psm.tile([P, 2, 2 * P], f32, tag="ph", bufs=2)
                        for m in range(2):
                            for k in range(3):
                                nc.tensor.matmul(
                                    ps_h[:, m, :],
                                    lhsT=w1[:, (k * 2 + m) * P:(k * 2 + m + 1) * P],
                                    rhs=cts[k][:],
                                    start=(k == 0), stop=(k == 2),
                                )
                        hs = mpool.tile([P, 2, 2 * P], f32r, tag="hs")
                        for m in range(2):
                            nc.scalar.activation(
                                hs[:, m, :], ps_h[:, m, :],
                                Act.Relu, bias=b1[:, m:m + 1],
                            )
                        ps_o = psm.tile([P, 2 * P], f32, tag="po", bufs=1)
                        for k in range(2):
                            nc.tensor.matmul(
                                ps_o[:], lhsT=w2[:, k * P:(k + 1) * P],
                                rhs=hs[:, k, :], start=(k == 0), stop=False,
                            )
                        for k in range(3):
                            nc.tensor.matmul(
                                ps_o[:], lhsT=ws[:, k * P:(k + 1) * P],
                                rhs=cts[k][:], start=False, stop=(k == 2),
                            )
                        ot = mpool.tile([P, 2 * P], f32, tag="ot")
                        nc.scalar.activation(ot[:], ps_o[:], Act.Relu, bias=bo[:, 0:1])
                        nc.sync.dma_start(d_out[:, i0 * P:(i0 + 2) * P], ot[:])

    nc.compile()
    return nc


def host_prep(inputs, tt=TT):
    """Build the per-core input maps (layout-only host prep)."""
    nt = tt * P
    x_target = np.asarray(inputs["x_target"], np.float32)
    pos_target = np.asarray(inputs["pos_target"], np.float32)
    x_source = np.asarray(inputs["x_source"], np.float32)
    pos_source = np.asarray(inputs["pos_source"], np.float32)
    W1 = np.asarray(inputs["W1"], np.float32)
    b1 = np.asarray(inputs["b1"], np.float32)
    W2 = np.asarray(inputs["W2"], np.float32)
    b2 = np.asarray(inputs["b2"], np.float32)
    Ws = np.asarray(inputs["Ws"], np.float32)
    bs = np.asarray(inputs["bs"], np.float32)

    w1t = W1.reshape(3, P, 2, P).transpose(1, 0, 2, 3).reshape(P, 3 * 2 * P).copy()
    w2t = W2.reshape(2, P, P).transpose(1, 0, 2).reshape(P, 2 * P).copy()
    wst = Ws.reshape(3, P, P).transpose(1, 0, 2).reshape(P, 3 * P).copy()
    b1t = b1.reshape(2, P).T.copy()
    bot = (b2 + bs).reshape(P, 1).copy()
    ident = np.eye(P, dtype=np.float32)

    in_maps = []
    for c in range(B):
        pt = pos_target[c * NT:c * NT + nt]
        ps = pos_source[c * NS:(c + 1) * NS]
        a_hi, a_lo = _bf16_split(pt)
        b_hi, b_lo = _bf16_split(ps)
        q = -0.5 * (ps.astype(np.float64) ** 2).sum(-1)
        q = q.astype(np.float32)
        q_hi, q_lo = _bf16_split(q)
        one = np.ones(nt, ml_dtypes.bfloat16)
        zero = np.zeros(nt, ml_dtypes.bfloat16)
        lhsT = np.stack(
            [a_hi[:, 0], a_hi[:, 0], a_lo[:, 0],
             a_hi[:, 1], a_hi[:, 1], a_lo[:, 1],
             a_hi[:, 2], a_hi[:, 2], a_lo[:, 2],
             one, one, zero], axis=0)
        zs = np.zeros(NS, ml_dtypes.bfloat16)
        rhs = np.stack(
            [b_hi[:, 0], b_lo[:, 0], b_hi[:, 0],
             b_hi[:, 1], b_lo[:, 1], b_hi[:, 1],
             b_hi[:, 2], b_lo[:, 2], b_hi[:, 2],
             q_hi, q_lo, zs], axis=0)
        ptT = pt.reshape(tt, P, 3).transpose(1, 0, 2).reshape(P, tt * 3).copy()
        nptq = (-0.5 * (pt.astype(np.float32) ** 2).sum(-1, dtype=np.float32)).reshape(tt, P).T.copy()
        xtT = x_target[c * NT:c * NT + nt].T.copy()
        ps_pad = np.zeros((NS, 64), np.float32)
        ps_pad[:, :3] = ps
        xs = x_source[c * NS:(c + 1) * NS].copy()
        in_maps.append({
            "lhsT_pt": lhsT, "rhs_ps": rhs, "ptT": ptT, "nptq": nptq, "xtT": xtT,
            "ps_pad": ps_pad, "xs": xs,
            "w1t": w1t, "w2t": w2t, "wst": wst, "b1t": b1t, "bot": bot,
            "ident": ident,
        })
    return in_maps


_CACHED = {}
LAST_RESULT = None


def kernel(**inputs):
    global LAST_RESULT
    from concourse import bass_utils

    if "nc" not in _CACHED:
        _CACHED["nc"] = build_program(TT)
    nc = _CACHED["nc"]
    in_maps = host_prep(inputs, TT)
    res = bass_utils.run_bass_kernel_spmd(nc, in_maps, core_ids=list(range(B)))
    LAST_RESULT = res
    outs = []
    for c in range(B):
        outT = res.results[c]["outT"]
        outs.append(np.ascontiguousarray(outT.T))
    return np.concatenate(outs, axis=0)
